# Initial kernel scaffold
#
"""Trainium2 kernel for nn_ContrastiveSSL: contrastive logits over sampled negatives.

Per sample n (one NeuronCore each, data-parallel over N=8):
  D[l, j]   = dot(c[:, l+1], z[:, j]) / ||z[:, j]||        (PE matmul, fp32)
  out[l, k] = D[l, full_inds[l, k]] / ||c[:, l+1]|| / TEMP  (per-row gather)
with full_inds[l] = [l, neg_inds[l, 0..99]] (distinct indices per row).

The per-row gather runs on GPSIMD via local_scatter: the host precomputes the
inverse index map (for each source column j, which output slot it feeds, or -1),
so the device scatters quantized D values straight into output slots.
D is scaled by 2*QSCALE/||c|| (folds the TEMP=0.5 divide) and quantized to
int16 (|logit| <= 2/TEMP = 4 bounds the value by 4*QSCALE = 32768), giving
~1e-4 absolute error relative to logit absmax.
"""

import sys

for _p in ("/opt/trn_rl_repo", "/root/.axon_site/_ro/trn_rl_repo"):
    if _p not in sys.path:
        sys.path.append(_p)

import numpy as np

N, C, L, K = 8, 128, 1024, 100
TEMP = 0.5
N_CORES = 8
G = 2                 # blocks of 128 rows merged per local_scatter call
NT = 8 // G           # number of scatter calls
QSCALE = 8192.0       # int16 fixed-point scale for quantized logits
MM_DTYPE = "float32"  # matmul input dtype: float32 (exact) or float32r (fast)

_CACHE = {}


def _build_program():
    import concourse.bacc as bacc
    import concourse.tile as tile
    import concourse.mybir as mybir
    from concourse.alu_op_type import AluOpType

    f32 = mybir.dt.float32
    i16 = mybir.dt.int16
    mmdt = getattr(mybir.dt, MM_DTYPE)

    nc = bacc.Bacc("TRN2", target_bir_lowering=False, debug=False,
                   num_devices=N_CORES)
    z_d = nc.dram_tensor("z", [C, L], f32, kind="ExternalInput").ap()
    cu_d = nc.dram_tensor("cu", [C, L], f32, kind="ExternalInput").ap()
    idx_d = nc.dram_tensor("idx", [C, 8 * L], i16, kind="ExternalInput").ap()
    onesc_d = nc.dram_tensor("onesc", [C, 1], f32, kind="ExternalInput").ap()
    ident_d = nc.dram_tensor("ident", [C, C], f32, kind="ExternalInput").ap()
    sel_d = nc.dram_tensor("sel", [8, L], f32, kind="ExternalInput").ap()
    out_d = nc.dram_tensor("out", [L, K + 1], f32, kind="ExternalOutput").ap()

    with tile.TileContext(nc) as tc:
        with (
            tc.tile_pool(name="consts", bufs=1) as cpool,
            tc.tile_pool(name="big", bufs=1) as bpool,
            tc.tile_pool(name="work", bufs=2) as wpool,
        ):
            ones_c = cpool.tile([C, 1], f32)
            ident = cpool.tile([C, C], f32)
            sel = cpool.tile([8, L], f32)
            nc.sync.dma_start(out=ones_c[:], in_=onesc_d[:])
            nc.sync.dma_start(out=ident[:], in_=ident_d[:])
            nc.sync.dma_start(out=sel[:], in_=sel_d[:])

            z_s = bpool.tile([C, L], f32, tag="z")
            cu_s = bpool.tile([C, L], f32, tag="cu")
            idx_s = bpool.tile([C, 8 * L], i16, tag="idx")
            nc.sync.dma_start(out=z_s[:], in_=z_d[:])
            nc.sync.dma_start(out=cu_s[:], in_=cu_d[:])
            nc.sync.dma_start(out=idx_s[:], in_=idx_d[:])

            # squared sums (ACT for z, DVE for c — parallel engines)
            zsq = bpool.tile([C, L], f32, tag="zsq")
            csq = bpool.tile([C, L], f32, tag="csq")
            nc.scalar.activation(zsq[:], z_s[:],
                                 mybir.ActivationFunctionType.Square)
            nc.vector.tensor_tensor(csq[:], cu_s[:], cu_s[:], op=AluOpType.mult)

            rcnq = wpool.tile([C, 8], f32, tag="rcnq")
            rzn_sb = bpool.tile([C, L], f32, tag="rznsb")

            if MM_DTYPE == "float32":
                cu_mm, z_mm = cu_s, z_s
            else:
                cu_mm = bpool.tile([C, L], mmdt, tag="cumm")
                z_mm = bpool.tile([C, L], mmdt, tag="zmm")
                nc.vector.tensor_copy(cu_mm[:], cu_s[:])
                nc.vector.tensor_copy(z_mm[:], z_s[:])

            # per merged group t: D matmuls, fused scale+quantize, scatter, out
            with tc.tile_pool(name="psD", bufs=2, space="PSUM") as psD:
                # t=0 D matmuls first so PE isn't stalled behind the
                # rzn transpose/broadcast chain below.
                dtiles = {}
                for b in range(G):
                    dps = psD.tile([C, L], f32, tag="dps")
                    dtiles[b] = dps
                    for h in range(2):
                        nc.tensor.matmul(dps[:, h * 512:(h + 1) * 512],
                                         cu_mm[:, b * C:(b + 1) * C],
                                         z_mm[:, h * 512:(h + 1) * 512],
                                         start=True, stop=True)

                # column-form squared norms: x_blk^T @ ones -> (128, 1) per block.
                # Column form keeps sqrt/reciprocal on all 128 partitions (a row-form
                # (1, 1024) reciprocal costs 6.5us on one partition).
                rz_col = wpool.tile([C, 8], f32, tag="rzcol")
                with tc.tile_pool(name="psS", bufs=1, space="PSUM") as psS:
                    zn2c = psS.tile([C, 8], f32, tag="zn2c")
                    cn2c = psS.tile([C, 8], f32, tag="cn2c")
                    for b in range(8):
                        nc.tensor.matmul(zn2c[:, b:b + 1], zsq[:, b * C:(b + 1) * C],
                                         ones_c[:], start=True, stop=True)
                    for b in range(8):
                        nc.tensor.matmul(cn2c[:, b:b + 1], csq[:, b * C:(b + 1) * C],
                                         ones_c[:], start=True, stop=True)
                    zsc = wpool.tile([C, 8], f32, tag="zsc8")
                    nc.scalar.activation(zsc[:], zn2c[:],
                                         mybir.ActivationFunctionType.Sqrt)
                    nc.vector.reciprocal(rz_col[:], zsc[:])
                    # rcn_q = 2*QSCALE/sqrt(cn2) = 1/sqrt(cn2 / (2*QSCALE)^2)
                    cnsc = wpool.tile([C, 8], f32, tag="cnsc")
                    nc.scalar.activation(cnsc[:], cn2c[:],
                                         mybir.ActivationFunctionType.Sqrt,
                                         scale=1.0 / float((2.0 * QSCALE) ** 2))
                    nc.vector.reciprocal(rcnq[:], cnsc[:])

                # rzn broadcast: PE-transpose rz_col to (8, 128) rows, then
                # replicate row b across all partitions via a selection matmul
                # (SEL[:, b*128:...] = e_b repeated, so SEL_b^T @ rz8 = row b
                # in every output partition; all operands at base partition 0).
                with tc.tile_pool(name="psB", bufs=1, space="PSUM") as psB:
                    tr8 = psB.tile([8, C], f32, tag="tr8")
                    nc.tensor.transpose(tr8[:], rz_col[:], ident[:])
                    rz8 = wpool.tile([8, C], f32, tag="rz8")
                    nc.vector.tensor_copy(rz8[:], tr8[:])
                    rzn_bc = psB.tile([C, L], f32, tag="rznbc")
                    for b in range(8):
                        nc.tensor.matmul(rzn_bc[:, b * C:(b + 1) * C],
                                         sel[:, b * C:(b + 1) * C], rz8[:],
                                         start=True, stop=True)
                    nc.scalar.copy(rzn_sb[:], rzn_bc[:])

                for t in range(NT):
                    dq = wpool.tile([C, G * L], i16, tag="dq")
                    for s_i in range(G):
                        b = G * t + s_i
                        if b in dtiles:
                            dps = dtiles.pop(b)
                        else:
                            dps = psD.tile([C, L], f32, tag="dps")
                            for h in range(2):
                                nc.tensor.matmul(
                                    dps[:, h * 512:(h + 1) * 512],
                                    cu_mm[:, b * C:(b + 1) * C],
                                    z_mm[:, h * 512:(h + 1) * 512],
                                    start=True, stop=True)
                        # quantize: int16(D * rcn_q[p, b] * rzn[j])
                        nc.vector.scalar_tensor_tensor(
                            dq[:, s_i * L:(s_i + 1) * L], dps[:],
                            rcnq[:, b:b + 1], rzn_sb[:],
                            op0=AluOpType.mult, op1=AluOpType.mult)
                    gath = wpool.tile([C, 102 * G], i16, tag="gath")
                    nc.gpsimd.local_scatter(gath[:], dq[:],
                                            idx_s[:, t * G * L:(t + 1) * G * L],
                                            channels=C, num_elems=102 * G,
                                            num_idxs=G * L)
                    for s_i in range(G):
                        b = G * t + s_i
                        ob = wpool.tile([C, K + 1], f32, tag="ob")
                        nc.scalar.activation(ob[:],
                                             gath[:, 102 * s_i:102 * s_i + K + 1],
                                             mybir.ActivationFunctionType.Copy,
                                             scale=1.0 / QSCALE)
                        nc.sync.dma_start(out=out_d[b * C:(b + 1) * C, :],
                                          in_=ob[:])

    nc.compile()
    return nc


def _host_prep(z, c, neg_inds):
    """Per-core input maps."""
    z = np.ascontiguousarray(z, dtype=np.float32)
    c = np.ascontiguousarray(c, dtype=np.float32)
    ni = np.asarray(neg_inds)
    ones_c = np.ones((C, 1), np.float32)
    in_maps = []
    ar = np.arange(L, dtype=np.int64)
    slot_base = np.arange(K + 1, dtype=np.int16)[None, :]  # (1, 101)
    seg = ((ar // C) % G).astype(np.int16)                 # (1024,)
    for n in range(N):
        fi = np.concatenate([ar[:, None], ni[n].astype(np.int64)], axis=1)
        vals = slot_base + (102 * seg)[:, None].astype(np.int16)  # (1024, 101)
        arr = np.full((L, L), -1, np.int16)
        np.put_along_axis(arr, fi, vals, axis=1)
        # device layout: (128, NT * G * 1024), partition p = row l % 128
        idx_dev = np.ascontiguousarray(
            arr.reshape(NT, G, C, L).transpose(2, 0, 1, 3).reshape(C, 8 * L))
        in_maps.append({
            "z": np.ascontiguousarray(z[n]),
            "cu": np.ascontiguousarray(c[n][:, 1:]),
            "idx": idx_dev,
            "onesc": ones_c,
            "ident": np.eye(C, dtype=np.float32),
            "sel": np.repeat(np.eye(8, dtype=np.float32), C, axis=1),
        })
    return in_maps


def kernel(z, c, neg_inds):
    from concourse import bass_utils

    if "nc" not in _CACHE:
        _CACHE["nc"] = _build_program()
    nc = _CACHE["nc"]
    in_maps = _host_prep(z, c, neg_inds)
    res = bass_utils.run_bass_kernel_spmd(nc, in_maps, core_ids=list(range(N_CORES)))
    out = np.concatenate([res.results[i]["out"] for i in range(N_CORES)], axis=0)
    return out.astype(np.float32)



# revision 22
# speedup vs baseline: 1.1912x; 1.1912x over previous
"""Trainium2 kernel for nn_ContrastiveSSL: contrastive logits over sampled negatives.

Per sample n (one NeuronCore each, data-parallel over N=8):
  D[l, j]   = dot(c[:, l+1], zn[:, j])   with zn = z / ||z_j||   (PE matmul, fp32r)
  out[l, k] = (D[l, fi[l, k]] * rcnq[l] + 128 - 128) / QS8        (per-row gather)
with fi[l] = [l, neg_inds[l, 0..99]] (distinct indices per row).

Gather strategy (GPSIMD local_scatter with uint8 pair-packing):
  - D rows are quantized to uint8 (q = D*rcnq[p] + 128, rcnq = QS8*2/||c_l||),
    packed 2-per-int16 unit, halving the scatter's (data, idx) stream vs int16.
  - Host precomputes the inverse unit map (unit u of row l -> output slot or -1).
    Since fi entries are distinct, a unit holds at most 2 targets; the second
    target of a colliding unit is routed by a tiny second scatter reading the
    first scatter's output, then merged by a DVE add (disjoint nonzeros).
  - The wanted byte is selected by two strided-uint8 activations (lo/hi byte
    planes -> f32 with scale 1/QS8 and bias -128/QS8) plus one copy_predicated
    on a host-provided parity mask.
"""

import sys

for _p in ("/opt/trn_rl_repo", "/root/.axon_site/_ro/trn_rl_repo"):
    if _p not in sys.path:
        sys.path.append(_p)

import numpy as np
import ml_dtypes

N, C, L, K = 8, 128, 1024, 100
TEMP = 0.5
N_CORES = 8
NB = 8                  # row blocks of 128 per sample
U = L // 2              # int16 units per packed row
SLOTS = K + 2           # output slots per row segment (101 used + pad)
SEGS = [1, 1, 2, 2, 2]  # blocks merged per scatter call (sum = NB)
QS8 = 120.0             # uint8 fixed-point scale for quantized logits
MM_DTYPE = "float32r"   # D-matmul input dtype

_CACHE = {}


def _build_program():
    import concourse.bacc as bacc
    import concourse.tile as tile
    import concourse.mybir as mybir
    from concourse.alu_op_type import AluOpType

    f32 = mybir.dt.float32
    bf16 = mybir.dt.bfloat16
    i16 = mybir.dt.int16
    u8 = mybir.dt.uint8
    mmdt = getattr(mybir.dt, MM_DTYPE)
    AF = mybir.ActivationFunctionType

    n_aux = NB * SLOTS  # per-plane aux (idx2 | par) length

    nc = bacc.Bacc("TRN2", target_bir_lowering=False, debug=False,
                   num_devices=N_CORES)
    z_d = nc.dram_tensor("z", [C, L], f32, kind="ExternalInput").ap()
    cu_d = nc.dram_tensor("cu", [C, L], f32, kind="ExternalInput").ap()
    idx1_d = nc.dram_tensor("idx1", [C, NB * U], i16, kind="ExternalInput").ap()
    aux_d = nc.dram_tensor("aux", [C, 2 * n_aux], i16, kind="ExternalInput").ap()
    ident_d = nc.dram_tensor("ident", [C, C], f32, kind="ExternalInput").ap()
    sel_d = nc.dram_tensor("sel", [8, L], bf16, kind="ExternalInput").ap()
    out_d = nc.dram_tensor("out", [L, K + 1], f32, kind="ExternalOutput").ap()

    with tile.TileContext(nc) as tc:
        with (
            tc.tile_pool(name="consts", bufs=1) as cpool,
            tc.tile_pool(name="big", bufs=1) as bpool,
            tc.tile_pool(name="work", bufs=2) as wpool,
        ):
            # small consts first (they gate the transpose/broadcast chain),
            # then z (heads the norm chain), cu, first idx chunk, rest.
            # input DMAs spread across per-engine queues so transfers overlap;
            # z first on sync: it heads the norm->quantize->scatter chain
            z_s = bpool.tile([C, L], f32, tag="z")
            cu_s = bpool.tile([C, L], f32, tag="cu")
            nc.sync.dma_start(out=z_s[:], in_=z_d[:])
            ident = cpool.tile([C, C], f32, tag="ident")
            sel = cpool.tile([8, L], bf16, tag="sel")
            nc.scalar.dma_start(out=ident[:], in_=ident_d[:])
            nc.scalar.dma_start(out=sel[:], in_=sel_d[:])
            nc.gpsimd.dma_start(out=cu_s[:], in_=cu_d[:])
            idx1_s = bpool.tile([C, NB * U], i16, tag="idx1")
            g0 = SEGS[0]
            nc.gpsimd.dma_start(out=idx1_s[:, :g0 * U], in_=idx1_d[:, :g0 * U])
            aux_s = bpool.tile([C, 2 * n_aux], i16, tag="aux")
            nc.scalar.dma_start(out=aux_s[:], in_=aux_d[:])
            nc.sync.dma_start(out=idx1_s[:, g0 * U:], in_=idx1_d[:, g0 * U:])
            idx2_s = aux_s[:, :n_aux]
            par_s = aux_s[:, n_aux:]
            ones_c = cpool.tile([C, 1], bf16, tag="ones")
            nc.vector.memset(ones_c[:], 1.0)

            # squared sums in bf16 (cheap PE weight loads; norm err ~0.05%)
            zsq = bpool.tile([C, L], bf16, tag="zsq")
            csq = bpool.tile([C, L], bf16, tag="csq")
            nc.scalar.activation(zsq[:], z_s[:], AF.Square)
            nc.vector.tensor_tensor(csq[:], cu_s[:], cu_s[:], op=AluOpType.mult)
            cu_b = bpool.tile([C, L], bf16, tag="cub")
            nc.vector.tensor_copy(cu_b[:], cu_s[:])

            # column-form norms: sq_blk^T @ ones -> (128, 1) per 128-col block
            rz_col = wpool.tile([C, 8], f32, tag="rzcol")
            rcnq = wpool.tile([C, 8], f32, tag="rcnq")
            zn = bpool.tile([C, L], bf16, tag="zn")
            with tc.tile_pool(name="psN", bufs=1, space="PSUM") as psN:
                zn2c = psN.tile([C, 8], f32, tag="zn2c")
                cn2c = psN.tile([C, 8], f32, tag="cn2c")
                for b in range(8):
                    nc.tensor.matmul(zn2c[:, b:b + 1], zsq[:, b * C:(b + 1) * C],
                                     ones_c[:], start=True, stop=True)
                for b in range(8):
                    nc.tensor.matmul(cn2c[:, b:b + 1], csq[:, b * C:(b + 1) * C],
                                     ones_c[:], start=True, stop=True)
                zsc = wpool.tile([C, 8], f32, tag="zsc8")
                nc.scalar.activation(zsc[:], zn2c[:], AF.Sqrt)
                nc.vector.reciprocal(rz_col[:], zsc[:])
                # rcnq = 2*QS8/sqrt(cn2) = 1/sqrt(cn2 / (2*QS8)^2)
                cnsc = wpool.tile([C, 8], f32, tag="cnsc")
                nc.scalar.activation(cnsc[:], cn2c[:], AF.Sqrt,
                                     scale=1.0 / float((2.0 * QS8) ** 2))
                nc.vector.reciprocal(rcnq[:], cnsc[:])

            # rzn broadcast to (C, L) rows, then zn = z * rzn (column-normalize z)
            with tc.tile_pool(name="psB", bufs=1, space="PSUM") as psB:
                tr8 = psB.tile([8, C], f32, tag="tr8")
                nc.tensor.transpose(tr8[:], rz_col[:], ident[:])
                rz8 = wpool.tile([8, C], bf16, tag="rz8")
                nc.scalar.copy(rz8[:], tr8[:])
                rzn_bc = psB.tile([C, L], f32, tag="rznbc")
                for b in range(8):
                    nc.tensor.matmul(rzn_bc[:, b * C:(b + 1) * C],
                                     sel[:, b * C:(b + 1) * C], rz8[:],
                                     start=True, stop=True)
                nc.vector.tensor_tensor(zn[:], z_s[:], rzn_bc[:],
                                        op=AluOpType.mult)

            # phase A: D matmuls + uint8 quantize, one dq tile per scatter call
            dq_tiles = []
            with tc.tile_pool(name="psD", bufs=2, space="PSUM") as psD:
                B0 = 0
                for ci, g in enumerate(SEGS):
                    dqc = bpool.tile([C, g * L], u8, tag=f"dq{ci}")
                    dq_tiles.append(dqc)
                    for s in range(g):
                        b = B0 + s
                        dps = psD.tile([C, L], f32, tag="dps")
                        for h in range(2):
                            nc.tensor.matmul(dps[:, h * 512:(h + 1) * 512],
                                             cu_b[:, b * C:(b + 1) * C],
                                             zn[:, h * 512:(h + 1) * 512],
                                             start=True, stop=True)
                        # q = D * rcnq[p] + 128, rounded into uint8
                        nc.scalar.activation(dqc[:, s * L:(s + 1) * L], dps[:],
                                             AF.Copy, scale=rcnq[:, b:b + 1],
                                             bias=128.0)
                    B0 += g

                # phase B: per scatter call: route units, fix collisions, unpack
                B0 = 0
                for ci, g in enumerate(SEGS):
                    gth1 = wpool.tile([C, g * SLOTS], i16, tag=f"gth1_{g}")
                    nc.gpsimd.local_scatter(
                        gth1[:], dq_tiles[ci][:].bitcast(i16),
                        idx1_s[:, B0 * U:(B0 + g) * U],
                        channels=C, num_elems=g * SLOTS, num_idxs=g * U)
                    gth2 = wpool.tile([C, g * SLOTS], i16, tag=f"gth2_{g}")
                    nc.gpsimd.local_scatter(
                        gth2[:], gth1[:],
                        idx2_s[:, B0 * SLOTS:(B0 + g) * SLOTS],
                        channels=C, num_elems=g * SLOTS, num_idxs=g * SLOTS)
                    xm = wpool.tile([C, g * SLOTS], i16, tag=f"xm_{g}")
                    nc.vector.tensor_tensor(xm[:], gth1[:], gth2[:],
                                            op=AluOpType.add)
                    # byte planes of the packed units, viewed at stride 2
                    xm8 = xm[:].bitcast(u8).rearrange("p (sl two) -> p sl two",
                                                      two=2)
                    ob = wpool.tile([C, g * (K + 1)], f32, tag=f"ob_{g}")
                    for s in range(g):
                        sl_lo = xm8[:, s * SLOTS:s * SLOTS + K + 1, 0]
                        sl_hi = xm8[:, s * SLOTS:s * SLOTS + K + 1, 1]
                        obs = ob[:, s * (K + 1):(s + 1) * (K + 1)]
                        nc.scalar.activation(obs, sl_lo, AF.Copy,
                                             scale=1.0 / QS8, bias=-128.0 / QS8)
                        hif = wpool.tile([C, K + 1], f32, tag="hif")
                        nc.scalar.activation(hif[:], sl_hi, AF.Copy,
                                             scale=1.0 / QS8, bias=-128.0 / QS8)
                        nc.vector.copy_predicated(
                            obs,
                            par_s[:, (B0 + s) * SLOTS:(B0 + s) * SLOTS + K + 1],
                            hif[:])
                    # one DMA per call: DRAM rows (B0+s)*128+p <- ob[p, s, :]
                    outv = out_d[B0 * C:(B0 + g) * C, :].rearrange(
                        "(s p) k -> p s k", p=C)
                    nc.sync.dma_start(out=outv, in_=ob[:])
                    B0 += g

    nc.compile()
    return nc


def _host_prep(z, c, neg_inds):
    """Per-core input maps (scatter unit maps, collision fixups, parities)."""
    z = np.ascontiguousarray(z, dtype=np.float32)
    c = np.ascontiguousarray(c, dtype=np.float32)
    ni = np.asarray(neg_inds)
    ar = np.arange(L, dtype=np.int64)
    in_maps = []
    for n in range(N):
        fi = np.concatenate([ar[:, None], ni[n].astype(np.int64)], axis=1)
        units = fi >> 1                                    # (L, K+1)
        par = (fi & 1).astype(np.int16)
        # stable-sort by unit; first occurrence is the primary target, an
        # adjacent equal unit is the (unique) secondary needing a fixup
        order = np.argsort(units, axis=1, kind="stable")
        us = np.take_along_axis(units, order, axis=1)
        dup = us[:, 1:] == us[:, :-1]
        un_map = np.full((L, U), -1, np.int16)
        first = np.ones(us.shape, bool)
        first[:, 1:] = ~dup
        rows_f, pos_f = np.nonzero(first)
        un_map[rows_f, us[rows_f, pos_f]] = order[rows_f, pos_f].astype(np.int16)
        rows, pos = np.nonzero(dup)
        idx2 = np.full((L, SLOTS), -1, np.int16)
        k_prim = order[rows, pos].astype(np.int16)
        k_sec = order[rows, pos + 1].astype(np.int16)
        idx2[rows, k_prim] = k_sec
        parm = np.zeros((L, SLOTS), np.int16)
        parm[:, :K + 1] = par
        # per-call slot base offsets (s = position of block within its call)
        sseg = np.zeros(NB, np.int16)
        B0 = 0
        for g in SEGS:
            sseg[B0:B0 + g] = np.arange(g, dtype=np.int16)
            B0 += g
        sl_base = (SLOTS * sseg[ar // C]).astype(np.int16)  # (L,)
        un_full = np.where(un_map >= 0, un_map + sl_base[:, None], -1)
        idx2_full = np.where(idx2 >= 0, idx2 + sl_base[:, None], -1)
        # device layout: partition p = l % 128, free = (block, unit/slot)
        idx1_dev = np.ascontiguousarray(
            un_full.astype(np.int16).reshape(NB, C, U)
            .transpose(1, 0, 2).reshape(C, NB * U))
        idx2_dev = (idx2_full.astype(np.int16).reshape(NB, C, SLOTS)
                    .transpose(1, 0, 2).reshape(C, NB * SLOTS))
        par_dev = (parm.reshape(NB, C, SLOTS)
                   .transpose(1, 0, 2).reshape(C, NB * SLOTS))
        aux_dev = np.ascontiguousarray(
            np.concatenate([idx2_dev, par_dev], axis=1))
        in_maps.append({
            "z": np.ascontiguousarray(z[n]),
            "cu": np.ascontiguousarray(c[n][:, 1:]),
            "idx1": idx1_dev,
            "aux": aux_dev,
            "ident": np.eye(C, dtype=np.float32),
            "sel": np.repeat(np.eye(8), C, axis=1).astype(ml_dtypes.bfloat16),
        })
    return in_maps


def kernel(z, c, neg_inds):
    from concourse import bass_utils

    if "nc" not in _CACHE:
        _CACHE["nc"] = _build_program()
    nc = _CACHE["nc"]
    in_maps = _host_prep(z, c, neg_inds)
    res = bass_utils.run_bass_kernel_spmd(nc, in_maps, core_ids=list(range(N_CORES)))
    out = np.concatenate([res.results[i]["out"] for i in range(N_CORES)], axis=0)
    return out.astype(np.float32)
